# revision 1
# baseline (speedup 1.0000x reference)
"""Trainium2 Bass kernel for LocalSingularityStrength (multi-scale box-filter
OLS slope + BN inference), data-parallel over 8 NeuronCores.

Algorithm per sample (H=224, W=224, C=32):
  1. xs = (x - min(x)) / (max(x) - min(x) + 1e-7)          (per-sample minmax)
  2. m_r = 2D box sum of xs with SAME padding, r in {2,4,8,16}
  3. alpha = OLS slope of ln(m_r + 1e-7) vs ln(r)
  4. out = (alpha - mu) / sqrt(var + 1e-3) * gamma + beta

Mapping:
  - batch is sharded 2 samples/core across 8 cores (pure data parallel).
  - each sample splits into 2 row-jobs of H/2 output rows (+<=7/8 halo rows),
    so a job fits the 128-partition SBUF layout [rows, (w,c)].
  - W-direction box sums: doubling shift-add cascade on VectorE in fp16.
  - H-direction box sums: banded all-ones matmuls on TensorE (band width <=16
    so contraction K<=128 covers a whole job), accumulating fp32 in PSUM.
  - ln(m + eps): ScalarE activation straight out of PSUM (evacuation fused).
  - OLS combine + BN: VectorE; per-channel BN vectors are precomputed on the
    host from gamma/beta/mean/var and folded to immediates when uniform.
"""

import math
import sys

sys.path.insert(0, "/opt/trn_rl_repo")

import numpy as np

import concourse.bacc as bacc
import concourse.bass as bass
import concourse.tile as tile
from concourse import mybir
from concourse.bass_utils import run_bass_kernel_spmd

FP16 = mybir.dt.float16
FP32 = mybir.dt.float32
ALU = mybir.AluOpType
ACT = mybir.ActivationFunctionType

NCORES = 8
SCALES = [2, 4, 8, 16]
NS = len(SCALES)
EPS_K = 1e-7
BN_EPS = 1e-3
PAD_L = 7
PAD_R = 9  # WP = W + 16

# OLS weights: alpha = sum_s q_s * ln(m_s), q_s = dls_s / den
_ls = np.log(np.array(SCALES, dtype=np.float64))
_dls = _ls - _ls.mean()
_den = float((_dls**2).sum())
Q16 = float(_dls[3] / _den)  # weight of ln(m16); q8 = Q16/3, q2 = -Q16, q4 = -Q16/3
C_RATIO = float(_dls[2] / _dls[3])  # exactly 1/3


def _job_rows(H):
    """Two output-row halves with input halo rows (window [-7, +8])."""
    M = H // 2
    jobs = []
    for a, b in ((0, M), (M, H)):
        lo_in = max(0, a - 7)
        hi_in = min(H - 1, b - 1 + 8)
        jobs.append((a, b, lo_in, hi_in - lo_in + 1))  # out_start, out_end, in_start, K
    return jobs


def _make_bands(H):
    """Banded all-ones lhsT matrices [K, M] per (job, scale), padded to 128 rows."""
    jobs = _job_rows(H)
    M = H // 2
    bands = np.zeros((2 * NS, 128, M), np.float16)
    for jt, (a, b, lo_in, K) in enumerate(jobs):
        for si, r in enumerate(SCALES):
            lo = (r - 1) // 2
            hi = r // 2
            for m in range(M):
                h = a + m
                r0 = max(0, h - lo)
                r1 = min(H - 1, h + hi)
                bands[jt * NS + si, r0 - lo_in : r1 - lo_in + 1, m] = 1.0
    return bands, jobs


def build_program(BS, H, W, C, n_cores=NCORES):
    """Build + compile the per-core bass program. Returns (nc, static inputs)."""
    assert H % 2 == 0 and H // 2 <= 120
    M = H // 2
    WP = W + PAD_L + PAD_R
    CW = 512 // C  # w-columns per output chunk
    assert 512 % C == 0 and W % CW == 0
    NCHUNK = W // CW

    bands_np, jobs = _make_bands(H)

    nc = bacc.Bacc("TRN2", target_bir_lowering=False, debug=False, num_devices=n_cores)
    x_in = nc.dram_tensor("x", [BS, H, W, C], FP32, kind="ExternalInput")
    bands_in = nc.dram_tensor("bands", list(bands_np.shape), FP16, kind="ExternalInput")
    scq_in = nc.dram_tensor("scq", [C], FP32, kind="ExternalInput")
    bi_in = nc.dram_tensor("bi", [C], FP32, kind="ExternalInput")
    out_t = nc.dram_tensor("out", [BS, H, W, C], FP32, kind="ExternalOutput")

    # The "uniform BN" fast path folds scq/bi to immediates baked into the
    # program; build_program is told which path to emit via module global set
    # by kernel() (see _BN_MODE). Default: uniform immediates.
    uniform, scq_imm, bi_imm = _BN_MODE

    with tile.TileContext(nc) as tc:
        with (
            tc.tile_pool(name="consts", bufs=1) as consts,
            tc.tile_pool(name="xraw", bufs=1) as xraw_pool,
            tc.tile_pool(name="wide", bufs=1) as wide,
            tc.tile_pool(name="small", bufs=4) as small,
            tc.tile_pool(name="chunks", bufs=3) as chunks,
            tc.tile_pool(name="outs", bufs=4) as outs,
            tc.tile_pool(name="psum", bufs=2, space="PSUM") as psum_pool,
        ):
            # ---- constants ----
            eps_sb = consts.tile([128, 1], FP32)
            nc.vector.memset(eps_sb, EPS_K)
            band_sb = consts.tile([128, 2 * NS, M], FP16)
            nc.sync.dma_start(
                out=band_sb,
                in_=bands_in.rearrange("s k m -> k s m"),
            )
            if not uniform:
                scq_sb = consts.tile([128, C], FP32)
                bi_sb = consts.tile([128, C], FP32)
                for dst, src in ((scq_sb, scq_in), (bi_sb, bi_in)):
                    nc.sync.dma_start(
                        out=dst,
                        in_=bass.AP(tensor=src.tensor, offset=0, ap=[[0, 128], [1, C]]),
                    )

            # ---- persistent working tiles ----
            xraw = [
                xraw_pool.tile([128, WP * C], FP32, name=f"xraw{i}", tag=f"xraw{i}")
                for i in range(2)
            ]
            xs = wide.tile([128, WP * C], FP16)
            wt = {
                r: wide.tile([128, WP * C], FP16, name=f"w{r}", tag=f"w{r}")
                for r in SCALES
            }
            # zero the pad columns of xs once (cascade reads them)
            nc.vector.memset(xs[:, 0 : PAD_L * C], 0.0)
            nc.vector.memset(xs[:, (PAD_L + W) * C : WP * C], 0.0)

            # per-sample scalar tiles
            def sample_scalars(b, jk):
                """Reduce min/max over this sample's two row-jobs -> inv, bias [128,1]."""
                armax = []
                armin = []
                for j in range(2):
                    P = jk[j][3]
                    xr = xraw[j][0:P, PAD_L * C : (PAD_L + W) * C]
                    rmax = small.tile([128, 1], FP32, tag="rmax")
                    rmin = small.tile([128, 1], FP32, tag="rmin")
                    nc.vector.tensor_reduce(
                        out=rmax[0:P], in_=xr.rearrange("p (w c) -> p w c", c=C),
                        axis=mybir.AxisListType.XY, op=ALU.max,
                    )
                    nc.vector.tensor_reduce(
                        out=rmin[0:P], in_=xr.rearrange("p (w c) -> p w c", c=C),
                        axis=mybir.AxisListType.XY, op=ALU.min,
                    )
                    rminn = small.tile([128, 1], FP32, tag="rminn")
                    nc.vector.tensor_scalar(
                        out=rminn[0:P], in0=rmin[0:P], scalar1=-1.0, scalar2=None,
                        op0=ALU.mult,
                    )
                    amax = small.tile([128, 1], FP32, tag="amax")
                    amin = small.tile([128, 1], FP32, tag="amin")
                    nc.gpsimd.partition_all_reduce(
                        amax[0:P], rmax[0:P], channels=P, reduce_op=bass.bass_isa.ReduceOp.max
                    )
                    nc.gpsimd.partition_all_reduce(
                        amin[0:P], rminn[0:P], channels=P, reduce_op=bass.bass_isa.ReduceOp.max
                    )
                    armax.append(amax)
                    armin.append(amin)
                mx = small.tile([128, 1], FP32, tag="mx")
                negmn = small.tile([128, 1], FP32, tag="negmn")
                nc.vector.tensor_tensor(out=mx[0:1], in0=armax[0][0:1], in1=armax[1][0:1], op=ALU.max)
                nc.vector.tensor_tensor(out=negmn[0:1], in0=armin[0][0:1], in1=armin[1][0:1], op=ALU.max)
                rng = small.tile([128, 1], FP32, tag="rng")
                nc.vector.tensor_tensor(out=rng[0:1], in0=mx[0:1], in1=negmn[0:1], op=ALU.add)
                rnge = small.tile([128, 1], FP32, tag="rnge")
                nc.vector.tensor_scalar(out=rnge[0:1], in0=rng[0:1], scalar1=EPS_K, scalar2=None, op0=ALU.add)
                inv1 = small.tile([128, 1], FP32, tag="inv1")
                nc.vector.reciprocal(out=inv1[0:1], in_=rnge[0:1])
                bia1 = small.tile([128, 1], FP32, tag="bia1")
                nc.vector.tensor_tensor(out=bia1[0:1], in0=negmn[0:1], in1=inv1[0:1], op=ALU.mult)
                inv_b = small.tile([128, 1], FP32, tag="inv_b")
                bia_b = small.tile([128, 1], FP32, tag="bia_b")
                nc.gpsimd.partition_broadcast(inv_b, inv1[0:1], channels=128)
                nc.gpsimd.partition_broadcast(bia_b, bia1[0:1], channels=128)
                return inv_b, bia_b

            for b in range(BS):
                jk = jobs
                # load both row-jobs of this sample
                for j, (a0, b0, lo_in, K) in enumerate(jk):
                    nc.sync.dma_start(
                        out=xraw[j][0:K, PAD_L * C : (PAD_L + W) * C],
                        in_=x_in[b, lo_in : lo_in + K].rearrange("k w c -> k (w c)"),
                    )
                inv_b, bia_b = sample_scalars(b, jk)

                for j, (a0, b0, lo_in, K) in enumerate(jk):
                    # normalize + cast to fp16
                    nc.vector.tensor_scalar(
                        out=xs[0:K, PAD_L * C : (PAD_L + W) * C],
                        in0=xraw[j][0:K, PAD_L * C : (PAD_L + W) * C],
                        scalar1=inv_b[0:K],
                        scalar2=bia_b[0:K],
                        op0=ALU.mult,
                        op1=ALU.add,
                    )
                    # W-direction doubling cascade (aligned to SAME padding)
                    def shift_add(dst, src, w0, w1, d0, d1):
                        # dst[w'] = src[w'+d0] + src[w'+d1] over w' in [w0, w1)
                        nc.vector.tensor_tensor(
                            out=dst[0:K, w0 * C : w1 * C],
                            in0=src[0:K, (w0 + d0) * C : (w1 + d0) * C],
                            in1=src[0:K, (w0 + d1) * C : (w1 + d1) * C],
                            op=ALU.add,
                        )

                    shift_add(wt[2], xs, 0, WP - 1, 0, 1)
                    shift_add(wt[4], wt[2], 1, WP - 2, -1, 1)
                    shift_add(wt[8], wt[4], 3, WP - 5, -2, 2)
                    shift_add(wt[16], wt[8], 7, PAD_L + W, -4, 4)

                    # H-direction banded matmuls + ln + OLS + BN per 512-chunk
                    for ci in range(NCHUNK):
                        cw0 = (PAD_L + ci * CW) * C
                        ps = psum_pool.tile([M, NS, 512], FP32)
                        for si, r in enumerate(SCALES):
                            nc.tensor.matmul(
                                ps[:, si, :],
                                lhsT=band_sb[0:K, j * NS + si, :],
                                rhs=wt[r][0:K, cw0 : cw0 + 512],
                                start=True,
                                stop=True,
                            )
                        lc = chunks.tile([M, NS, 512], FP16, tag="lc")
                        nc.scalar.activation(
                            out=lc, in_=ps, func=ACT.Ln, bias=eps_sb[0:M], scale=1.0
                        )
                        sp = chunks.tile([M, 2, 512], FP16, tag="sp")
                        # s1 = L8 - L4 ; s2 = L16 - L2
                        nc.vector.tensor_tensor(
                            out=sp[:, 0, :], in0=lc[:, 2, :], in1=lc[:, 1, :], op=ALU.subtract
                        )
                        nc.vector.tensor_tensor(
                            out=sp[:, 1, :], in0=lc[:, 3, :], in1=lc[:, 0, :], op=ALU.subtract
                        )
                        tmp = chunks.tile([M, 512], FP16, tag="tmp")
                        nc.vector.scalar_tensor_tensor(
                            out=tmp, in0=sp[:, 0, :], scalar=C_RATIO, in1=sp[:, 1, :],
                            op0=ALU.mult, op1=ALU.add,
                        )
                        oc = outs.tile([M, 512], FP32, tag="oc")
                        if uniform:
                            nc.vector.tensor_scalar(
                                out=oc, in0=tmp, scalar1=scq_imm, scalar2=bi_imm,
                                op0=ALU.mult, op1=ALU.add,
                            )
                        else:
                            m1 = chunks.tile([M, 512], FP32, tag="m1")
                            scq_ap = bass.AP(
                                tensor=scq_sb.tensor, offset=scq_sb.offset,
                                ap=[scq_sb.ap[0][:], [0, CW], [1, C]],
                            )
                            bi_ap = bass.AP(
                                tensor=bi_sb.tensor, offset=bi_sb.offset,
                                ap=[bi_sb.ap[0][:], [0, CW], [1, C]],
                            )
                            nc.vector.tensor_tensor(
                                out=m1.rearrange("p (w c) -> p w c", c=C),
                                in0=tmp.rearrange("p (w c) -> p w c", c=C),
                                in1=scq_ap[0:M], op=ALU.mult,
                            )
                            nc.vector.tensor_tensor(
                                out=oc.rearrange("p (w c) -> p w c", c=C),
                                in0=m1.rearrange("p (w c) -> p w c", c=C),
                                in1=bi_ap[0:M], op=ALU.add,
                            )
                        nc.sync.dma_start(
                            out=out_t[b, a0:b0, ci * CW : (ci + 1) * CW, :].rearrange(
                                "m w c -> m (w c)"
                            ),
                            in_=oc,
                        )

    nc.compile()
    return nc


# (uniform, scq_imm, bi_imm) — set by kernel() before build; default uniform
_BN_MODE = (True, Q16, 0.0)

_PROG_CACHE = {}


def _get_program(BS, H, W, C, bn_mode):
    key = (BS, H, W, C, bn_mode)
    if key not in _PROG_CACHE:
        global _BN_MODE
        _BN_MODE = bn_mode
        _PROG_CACHE[key] = build_program(BS, H, W, C)
    return _PROG_CACHE[key]


def kernel(x, gamma, beta, moving_mean, moving_var):
    x = np.asarray(x)
    gamma = np.asarray(gamma, dtype=np.float32)
    beta = np.asarray(beta, dtype=np.float32)
    moving_mean = np.asarray(moving_mean, dtype=np.float32)
    moving_var = np.asarray(moving_var, dtype=np.float32)

    B, H, W, C = x.shape
    assert B % NCORES == 0
    BS = B // NCORES

    # host-side BN folding: out = tmp * (q16*sc) + (beta - mean*sc)
    sc = gamma / np.sqrt(moving_var + np.float32(BN_EPS))
    scq = (sc * np.float32(Q16)).astype(np.float32)
    bi = (beta - moving_mean * sc).astype(np.float32)
    uniform = bool(np.ptp(scq) == 0 and np.ptp(bi) == 0)
    bn_mode = (uniform, float(scq[0]), float(bi[0])) if uniform else (False, 0.0, 0.0)

    nc = _get_program(BS, H, W, C, bn_mode)

    bands_np, _ = _make_bands(H)
    x_np = np.ascontiguousarray(x, dtype=np.float32)
    in_maps = []
    for i in range(NCORES):
        in_maps.append(
            {
                "x": x_np[i * BS : (i + 1) * BS],
                "bands": bands_np,
                "scq": scq,
                "bi": bi,
            }
        )
    res = run_bass_kernel_spmd(nc, in_maps, list(range(NCORES)))
    out = np.concatenate([res.results[i]["out"] for i in range(NCORES)], axis=0)
    return out.astype(np.float32)

